# revision 2
# baseline (speedup 1.0000x reference)
"""Trainium2 Bass kernel for nn_JunmaiLayer (gnn_message_passing).

Math: h[z,a,o] = sum_{b,d,e,k,c} basis[z,a,b,k,c] * basis[z,d,e,k,c] * W[a,b,d,e,k,o]
      out = silu(h) @ w_fc + b_fc

Factoring used here:
  G[z,k,ab,de] = sum_c basis[z,ab,k,c] * basis[z,de,k,c]      (tiny, host-computed)
  h[z,a,o]    = sum_{b,k,de} G[z,k,ab,de] * W[ab,de,k,o]      (device, streams all of W)

W is 256 MB fp32 and each element is used once. Under this axon client the
dominant cost of a dispatch is shipping W over the tunnel (~45 MB/s), so W
goes over the wire as int8 (symmetric linear quant, one global scale:
rel-to-max err ~7e-3 vs the 2e-2 gate; fp8 e4m3 measures 2-3e-2 and fails).
The scale is folded into the host epilogue (device works on integer-valued
fp16, PSUM accumulates fp32 exactly).

Sharding: W split along its leading atom axis `a` across 8 cores (2 atoms
each, 8 MB int8 per core). x/basis/G are replicated (G sliced per core).
Each core computes h[z, a_slice, o]; host concatenates (the "all-gather")
and applies the trivial silu+fc epilogue.

Device kernel per core:
  - DMA G slice (fp16, 1 MB) into SBUF once.
  - Stream W slice in 1 MB int8 chunks (4 ab-pairs worth), 8 chunks.
  - DVE converts each chunk int8 -> fp16 into a 2-deep ping-pong buffer
    (PE signals chunk completion on pe_prog so the buffer can be reused).
  - For each (ab, k, de-half): one matmul  psum[z=4, o=64] +=
        G[de128, z4].T @ Wf[de128, o64]   accumulated over 512 matmuls/atom.
  - DVE copies the two psum tiles [4,64] to SBUF, sync engine DMAs out
    h [2,4,64] fp32.
"""

import numpy as np

import concourse.bass as bass
import concourse.tile as tile
from concourse import mybir
from concourse.bass_utils import run_bass_kernel_spmd

# ---------------------------------------------------------------- constants
B, N, K, H, O = 4, 16, 16, 64, 1
EPSILON = 1e-5
CUT_LO, CUT_HI = 0.0, 5.0
N_CORES = 8
A_PER_CORE = N // N_CORES          # 2 atoms per core
AB_PER_CORE = A_PER_CORE * N       # 32 (a,b) pairs per core
DE = N * N                         # 256 contraction values, 2 chunks of 128
N_CHUNKS = 8                       # W stream chunks per core (4 ab each, 1 MB int8)
AB_PER_CHUNK = AB_PER_CORE // N_CHUNKS  # 4
CHUNK_F = AB_PER_CHUNK * 2 * K * H      # 8192 free-dim elements per chunk

_nc_cache = {}


def _basis_host(x):
    """Replicates reference featurization in float64; returns (B, N*N, K, 3)."""
    x = x.astype(np.float64)
    diff = x[:, :, None, :] - x[:, None, :, :]                # (B,N,N,3)
    norm_sq = np.sum(diff * diff, axis=-1, keepdims=True) + EPSILON
    norm = np.sqrt(norm_sq)
    diffn = diff / norm_sq
    start = np.exp(-CUT_HI + CUT_LO)
    means = np.linspace(start, 1.0, K)
    betas = (2.0 / K * (1.0 - start)) ** -2
    alpha = 5.0 / (CUT_HI - CUT_LO)
    cutoff = 0.5 * (np.cos(np.pi * norm / CUT_HI) + 1.0) * (norm < CUT_HI)
    smear = cutoff * np.exp(-betas * (np.exp(alpha * (-norm + CUT_LO)) - means) ** 2)
    basis = smear[..., None] * diffn[..., None, :]            # (B,N,N,K,3)
    return basis.reshape(B, N * N, K, 3)


def _build_nc():
    """One SPMD Bass program (raw Block API); every core runs its W/G slice.

    Pipeline: sync engine queues G + 8 int8 W-chunk DMAs back-to-back (HWDGE
    FIFO); DVE waits per-chunk (own semaphore each -- a shared counting sem
    can race across the 16 SDMA engines) and casts the chunk to fp16 into a
    ping-pong buffer; PE waits on the cast (conv_sem) and runs 128
    accumulating matmuls per chunk, signalling chunk completion on pe_prog
    (DVE reuses a ping-pong slot only after pe_prog covers it); DVE copies
    the two PSUM results to SBUF; sync engine DMAs them out.
    """
    nc = bass.Bass(target_bir_lowering=False)
    # Host pre-arranges W in exact SBUF layout: [p, chunk, q=ab_in_chunk*2+t, ko]
    # so every partition's read per chunk is one contiguous 8 KB block.
    w = nc.dram_tensor("w", [128, N_CHUNKS, CHUNK_F], mybir.dt.int8,
                       kind="ExternalInput")
    g = nc.dram_tensor("g", [128, AB_PER_CORE * K * 2 * B], mybir.dt.float16,
                       kind="ExternalInput")
    h = nc.dram_tensor("h", [B, A_PER_CORE * H], mybir.dt.float32,
                       kind="ExternalOutput")

    import contextlib
    with contextlib.ExitStack() as st:
        gt = st.enter_context(nc.sbuf_tensor(
            "gt", [128, AB_PER_CORE * K * 2 * B], mybir.dt.float16))
        wt8 = st.enter_context(nc.sbuf_tensor(
            "wt8", [128, N_CHUNKS, CHUNK_F], mybir.dt.int8))
        wf = st.enter_context(nc.sbuf_tensor(
            "wf", [128, 2, CHUNK_F], mybir.dt.float16))
        ot = st.enter_context(nc.sbuf_tensor(
            "ot", [B, A_PER_CORE * H], mybir.dt.float32))
        ps = [st.enter_context(nc.psum_tensor(f"ps{ai}", [B, H], mybir.dt.float32))
              for ai in range(A_PER_CORE)]
        g_sem = st.enter_context(nc.semaphore("g_sem"))
        w_sems = [st.enter_context(nc.semaphore(f"w_sem{cc}"))
                  for cc in range(N_CHUNKS)]
        conv_sem = st.enter_context(nc.semaphore("conv_sem"))
        pe_prog = st.enter_context(nc.semaphore("pe_prog"))
        cp_sem = st.enter_context(nc.semaphore("cp_sem"))
        out_sem = st.enter_context(nc.semaphore("out_sem"))
        block = st.enter_context(nc.Block())

        @block.sync
        def _(sync):
            for cc in range(N_CHUNKS):
                sync.dma_start(
                    wt8[:, cc, :], w[:, cc, :],
                ).then_inc(w_sems[cc], 16)
            # Output store rides the same HWDGE ring; by the time cp_sem
            # fires the W stream has long drained, so no queuing delay.
            sync.wait_ge(cp_sem, 1)
            sync.dma_start(h[:, :], ot[:, :]).then_inc(out_sem, 16)
            sync.wait_ge(out_sem, 16)

        @block.vector
        def _(vector):
            for cc in range(N_CHUNKS):
                vector.wait_ge(w_sems[cc], 16)
                if cc >= 2:
                    # ping-pong slot cc%2 is free once PE finished chunk cc-2
                    vector.wait_ge(pe_prog, cc - 1)
                vector.tensor_copy(
                    out=wf[:, cc % 2, :], in_=wt8[:, cc, :],
                ).then_inc(conv_sem, 1)
            for ai in range(A_PER_CORE):
                vector.wait_ge(pe_prog, (ai + 1) * (N_CHUNKS // A_PER_CORE))
                cp = vector.tensor_copy(
                    out=ot[:, ai * H:(ai + 1) * H], in_=ps[ai][:, :])
                if ai == A_PER_CORE - 1:
                    cp.then_inc(cp_sem, 1)

        @block.tensor
        def _(tensor):
            tensor.wait_ge(g_sem, 16)
            for cc in range(N_CHUNKS):
                tensor.wait_ge(conv_sem, cc + 1)
                for abin in range(AB_PER_CHUNK):
                    ab = cc * AB_PER_CHUNK + abin
                    ai = ab // N
                    for k in range(K):
                        for t in range(2):
                            col = ((ab * K + k) * 2 + t) * B
                            off = ((abin * 2 + t) * K + k) * H
                            stop = (ab % N == N - 1 and k == K - 1 and t == 1)
                            last = (abin == AB_PER_CHUNK - 1 and k == K - 1
                                    and t == 1)
                            mm = tensor.matmul(
                                ps[ai][:, :],
                                gt[:, col:col + B],
                                wf[:, cc % 2, off:off + H],
                                start=(ab % N == 0 and k == 0 and t == 0),
                                stop=stop,
                            )
                            if last:
                                # chunk completion doubles as psum-ready at
                                # cc=3 (atom 0) and cc=7 (atom 1)
                                mm.then_inc(pe_prog, 1)

        @block.gpsimd
        def _(gpsimd):
            # G load on the SWDGE path overlaps W chunk 0 on the HWDGE ring.
            gpsimd.dma_start(gt[:, :], g[:, :]).then_inc(g_sem, 16)
    return nc


def _get_nc():
    if "nc" not in _nc_cache:
        _nc_cache["nc"] = _build_nc()
    return _nc_cache["nc"]


def _quantize_w(W):
    """Symmetric linear int8 quant of W; returns (int8 array in the device
    layout's pre-slice form (N, N, DE, K*H), python float scale)."""
    wmax = float(np.abs(W).max())
    scale = wmax / 127.0 if wmax > 0 else 1.0
    Wq = np.rint(W.reshape(N, N, DE, K * H) * (1.0 / scale))
    np.clip(Wq, -127, 127, out=Wq)
    return Wq.astype(np.int8), scale


def _make_inputs(x, W):
    bf = _basis_host(x)                                        # (B, 256, K, 3)
    G = np.einsum("zikc,zjkc->zkij", bf, bf)                   # (B, K, 256, 256)
    W8, scale = _quantize_w(np.asarray(W))
    in_maps = []
    for c in range(N_CORES):
        wc = W8[c * A_PER_CORE:(c + 1) * A_PER_CORE].reshape(
            AB_PER_CORE, 2, 128, K * H)          # (ab, t, p, f)
        wc = wc.transpose(2, 0, 1, 3).reshape(128, N_CHUNKS, CHUNK_F)
        gc = G[:, :, c * AB_PER_CORE:(c + 1) * AB_PER_CORE, :]  # (B,K,32,256)
        gc = gc.reshape(B, K, AB_PER_CORE, 2, 128)
        gc = gc.transpose(4, 2, 1, 3, 0).reshape(128, AB_PER_CORE * K * 2 * B)
        in_maps.append({
            "w": np.ascontiguousarray(wc),
            "g": np.ascontiguousarray(gc, dtype=np.float16),
        })
    return in_maps, scale


def kernel(x, W, w_fc, b_fc):
    nc = _get_nc()
    in_maps, scale = _make_inputs(x, W)
    res = run_bass_kernel_spmd(nc, in_maps, list(range(N_CORES))).results
    h = np.zeros((B, N, H), dtype=np.float64)
    for c in range(N_CORES):
        hc = res[c]["h"].reshape(B, A_PER_CORE, H)             # (B, 2, H)
        for ai in range(A_PER_CORE):
            h[:, c * A_PER_CORE + ai, :] = hc[:, ai, :]
    h *= scale
    sil = h / (1.0 + np.exp(-h))
    out = sil @ w_fc.astype(np.float64) + b_fc.astype(np.float64)
    return out.astype(np.float32)


# revision 3
# speedup vs baseline: 2.9609x; 2.9609x over previous
"""Trainium2 Bass kernel for nn_JunmaiLayer (gnn_message_passing).

Math: h[z,a,o] = sum_{b,d,e,k,c} basis[z,a,b,k,c] * basis[z,d,e,k,c] * W[a,b,d,e,k,o]
      out = silu(h) @ w_fc + b_fc

Factoring:
  G[z,k,ab,de] = sum_c basis[z,ab,k,c] * basis[z,de,k,c]      (tiny, host-computed)
  h[z,a,o]    = sum_{b,k,de} G[z,k,ab,de] * W[ab,de,k,o]      (device)

Two structural facts make this far smaller than the dense 256 MB W stream:

1. The RBF cutoff zeroes basis rows EXACTLY for atom pairs with dist >= 5
   (and for a==b, where diff==0). With x ~ N(0,9) per coord only ~20% of
   (ab, de) cells have sum_z |G[z,:,ab,de]| != 0, so ~80% of W is never
   touched. The host computes the exact activity mask from x and ships only
   the needed W[ab,de,:,:] cells. This is exact sparsity, not approximation.

2. Under this axon client a dispatch is wire-bound (~45-60 MB/s tunnel), so
   W cells go over as int8 (symmetric linear quant, one global scale:
   rel-to-max err ~7e-3 vs the 2e-2 gate; fp8 e4m3 measures 2-3e-2). The
   scale folds into the host epilogue; the device matmuls integer-valued
   fp16 and PSUM accumulates fp32 exactly.

Packed layout: every needed (ab, de) cell contributes K=16 contraction rows
(one per k), each row = 64 int8 W values + an 8-wide fp16 G vector
(slot*4+z, slot = which of the core's two atoms the row belongs to; the
other slot is zero). Rows are tiled 128 to a matmul. Atoms are paired
greedy (largest+smallest row count) onto 8 cores; all cores run one SPMD
program sized to the max core (zero-padded tiles contribute nothing).

Device per core: stream W tiles in 8 int8 chunks (HWDGE, per-chunk
semaphores), DVE casts each chunk to fp16 into a 2-deep ping-pong buffer,
PE runs one accumulating matmul per tile: psum[8,64] += G[128,8].T @
Wf[128,64]; DVE copies psum to SBUF, sync engine DMAs out h[8,64] fp32.
Host scales by the quant scale, applies silu + fc, and scatters the 2
atoms per core into the full (B,N,O) output.
"""

import numpy as np

import concourse.bass as bass
from concourse import mybir
from concourse.bass_utils import run_bass_kernel_spmd

# ---------------------------------------------------------------- constants
B, N, K, H, O = 4, 16, 16, 64, 1
EPSILON = 1e-5
CUT_LO, CUT_HI = 0.0, 5.0
N_CORES = 8
N_CHUNKS = 8
SLOTS = 2                       # atoms per core
GW = SLOTS * B                  # G vector width (slot-major, z minor)

_nc_cache = {}


def _basis_host(x):
    """Replicates reference featurization in float64; returns (B, N*N, K, 3)."""
    x = x.astype(np.float64)
    diff = x[:, :, None, :] - x[:, None, :, :]                # (B,N,N,3)
    norm_sq = np.sum(diff * diff, axis=-1, keepdims=True) + EPSILON
    norm = np.sqrt(norm_sq)
    diffn = diff / norm_sq
    start = np.exp(-CUT_HI + CUT_LO)
    means = np.linspace(start, 1.0, K)
    betas = (2.0 / K * (1.0 - start)) ** -2
    alpha = 5.0 / (CUT_HI - CUT_LO)
    cutoff = 0.5 * (np.cos(np.pi * norm / CUT_HI) + 1.0) * (norm < CUT_HI)
    smear = cutoff * np.exp(-betas * (np.exp(alpha * (-norm + CUT_LO)) - means) ** 2)
    basis = smear[..., None] * diffn[..., None, :]            # (B,N,N,K,3)
    return basis.reshape(B, N * N, K, 3)


def _build_nc(nt8, ct):
    """One SPMD Bass program; nt8 = padded tile count, ct = tiles/chunk."""
    nc = bass.Bass(target_bir_lowering=False)
    w = nc.dram_tensor("w", [128, N_CHUNKS, ct * H], mybir.dt.int8,
                       kind="ExternalInput")
    g = nc.dram_tensor("g", [128, nt8 * GW], mybir.dt.float16,
                       kind="ExternalInput")
    h = nc.dram_tensor("h", [GW, H], mybir.dt.float32, kind="ExternalOutput")

    import contextlib
    with contextlib.ExitStack() as st:
        gt = st.enter_context(nc.sbuf_tensor(
            "gt", [128, nt8 * GW], mybir.dt.float16))
        wt8 = st.enter_context(nc.sbuf_tensor(
            "wt8", [128, N_CHUNKS, ct * H], mybir.dt.int8))
        wf = st.enter_context(nc.sbuf_tensor(
            "wf", [128, 2, ct * H], mybir.dt.float16))
        ot = st.enter_context(nc.sbuf_tensor("ot", [GW, H], mybir.dt.float32))
        ps = st.enter_context(nc.psum_tensor("ps", [GW, H], mybir.dt.float32))
        g_sem = st.enter_context(nc.semaphore("g_sem"))
        w_sems = [st.enter_context(nc.semaphore(f"w_sem{cc}"))
                  for cc in range(N_CHUNKS)]
        conv_sem = st.enter_context(nc.semaphore("conv_sem"))
        pe_prog = st.enter_context(nc.semaphore("pe_prog"))
        cp_sem = st.enter_context(nc.semaphore("cp_sem"))
        out_sem = st.enter_context(nc.semaphore("out_sem"))
        block = st.enter_context(nc.Block())

        @block.sync
        def _(sync):
            for cc in range(N_CHUNKS):
                # per-chunk semaphores: a shared counting sem can race
                # across the 16 SDMA engines
                sync.dma_start(wt8[:, cc, :], w[:, cc, :]).then_inc(
                    w_sems[cc], 16)
            sync.wait_ge(cp_sem, 1)
            sync.dma_start(h[:, :], ot[:, :]).then_inc(out_sem, 16)
            sync.wait_ge(out_sem, 16)

        @block.vector
        def _(vector):
            for cc in range(N_CHUNKS):
                vector.wait_ge(w_sems[cc], 16)
                if cc >= 2:
                    # ping-pong slot cc%2 is free once PE finished chunk cc-2
                    vector.wait_ge(pe_prog, cc - 1)
                vector.tensor_copy(
                    out=wf[:, cc % 2, :], in_=wt8[:, cc, :],
                ).then_inc(conv_sem, 1)
            vector.wait_ge(pe_prog, N_CHUNKS)
            vector.tensor_copy(out=ot[:, :], in_=ps[:, :]).then_inc(cp_sem, 1)

        @block.tensor
        def _(tensor):
            tensor.wait_ge(g_sem, 16)
            for cc in range(N_CHUNKS):
                tensor.wait_ge(conv_sem, cc + 1)
                for t in range(ct):
                    tile = cc * ct + t
                    mm = tensor.matmul(
                        ps[:, :],
                        gt[:, tile * GW:(tile + 1) * GW],
                        wf[:, cc % 2, t * H:(t + 1) * H],
                        start=(tile == 0),
                        stop=(tile == nt8 - 1),
                    )
                    if t == ct - 1:
                        mm.then_inc(pe_prog, 1)

        @block.gpsimd
        def _(gpsimd):
            # G load on the SWDGE path overlaps W chunk 0 on the HWDGE ring.
            gpsimd.dma_start(gt[:, :], g[:, :]).then_inc(g_sem, 16)
    return nc


def _get_nc(nt8, ct):
    key = (nt8, ct)
    if key not in _nc_cache:
        _nc_cache[key] = _build_nc(nt8, ct)
    return _nc_cache[key]


def _make_inputs(x, W):
    """Returns (in_maps, meta) where meta = (scale, pairs, nt8, ct)."""
    x = np.asarray(x)
    bf = _basis_host(x)                                       # (B, 256, K, 3)
    # G[z,k,i,j] via batched matmul: (z,k,i,c) @ (z,k,c,j)
    bkt = bf.transpose(0, 2, 1, 3)                            # (B, K, 256, 3)
    G = bkt @ bkt.transpose(0, 1, 3, 2)                       # (B, K, 256, 256)

    # exact activity mask from the cutoff (and the a==b zero-diff rows)
    xd = x.astype(np.float64)
    diff = xd[:, :, None, :] - xd[:, None, :, :]
    dist = np.sqrt((diff ** 2).sum(-1) + EPSILON)
    act = (dist < CUT_HI) & ~np.eye(N, dtype=bool)[None]      # (B, N, N)
    actf = act.reshape(B, N * N)
    need = np.zeros((N * N, N * N), dtype=bool)               # (ab, de)
    for z in range(B):
        need |= actf[z][:, None] & actf[z][None, :]

    Wv = np.asarray(W).reshape(N * N, N * N, K, H)

    # per-atom cell lists (ab = a*N + b, so atom-major already)
    atom_cells = []
    for a in range(N):
        b_idx, de_idx = np.nonzero(need[a * N:(a + 1) * N])
        atom_cells.append((a * N + b_idx, de_idx))
    rows_per_atom = np.array([len(ab) * K for ab, _ in atom_cells])

    # pair largest with smallest onto the 8 cores
    order = np.argsort(-rows_per_atom, kind="stable")
    pairs = [(int(order[i]), int(order[2 * N_CORES - 1 - i]))
             for i in range(N_CORES)]
    core_rows = [rows_per_atom[i] + rows_per_atom[j] for i, j in pairs]
    nt = max((int(r) + 127) // 128 for r in core_rows) if max(core_rows) else 1
    ct = (nt + N_CHUNKS - 1) // N_CHUNKS
    nt8 = ct * N_CHUNKS

    # quantization scale over the needed cells only
    all_ab = np.concatenate([c[0] for c in atom_cells])
    all_de = np.concatenate([c[1] for c in atom_cells])
    wmax = float(np.abs(Wv[all_ab, all_de]).max()) if len(all_ab) else 1.0
    scale = wmax / 127.0 if wmax > 0 else 1.0
    inv = 1.0 / scale

    def atom_rows(a):
        ab, de = atom_cells[a]
        wc = np.rint(Wv[ab, de].astype(np.float64) * inv)     # (nc, K, H)
        np.clip(wc, -127, 127, out=wc)
        wr = wc.astype(np.int8).reshape(-1, H)                # (nc*K, H)
        gc = G[:, :, ab, de]                                  # (B, K, nc)
        gr = gc.transpose(2, 1, 0).reshape(-1, B)             # (nc*K, B)
        return wr, gr.astype(np.float16)

    in_maps = []
    for c in range(N_CORES):
        a0, a1 = pairs[c]
        w0, g0 = atom_rows(a0)
        w1, g1 = atom_rows(a1)
        nrows = nt8 * 128
        wrows = np.zeros((nrows, H), dtype=np.int8)
        grows = np.zeros((nrows, GW), dtype=np.float16)
        wrows[:len(w0)] = w0
        wrows[len(w0):len(w0) + len(w1)] = w1
        grows[:len(g0), 0:B] = g0
        grows[len(g0):len(g0) + len(g1), B:2 * B] = g1
        wc = wrows.reshape(nt8, 128, H).transpose(1, 0, 2).reshape(
            128, N_CHUNKS, ct * H)
        gc = grows.reshape(nt8, 128, GW).transpose(1, 0, 2).reshape(
            128, nt8 * GW)
        in_maps.append({
            "w": np.ascontiguousarray(wc),
            "g": np.ascontiguousarray(gc),
        })
    return in_maps, (scale, pairs, nt8, ct)


def kernel(x, W, w_fc, b_fc):
    in_maps, (scale, pairs, nt8, ct) = _make_inputs(x, W)
    nc = _get_nc(nt8, ct)
    res = run_bass_kernel_spmd(nc, in_maps, list(range(N_CORES))).results
    h = np.zeros((B, N, H), dtype=np.float64)
    for c in range(N_CORES):
        hc = res[c]["h"].reshape(SLOTS, B, H)                 # (slot, z, H)
        for s in range(SLOTS):
            h[:, pairs[c][s], :] = hc[s]
    h *= scale
    sil = h / (1.0 + np.exp(-h))
    out = sil @ w_fc.astype(np.float64) + b_fc.astype(np.float64)
    return out.astype(np.float32)


# revision 4
# speedup vs baseline: 6.9938x; 2.3620x over previous
"""Trainium2 Bass kernel for nn_JunmaiLayer (gnn_message_passing).

Math: h[z,a,o] = sum_{b,d,e,k,c} basis[z,a,b,k,c] * basis[z,d,e,k,c] * W[a,b,d,e,k,o]
      out = silu(h) @ w_fc + b_fc

Factoring:
  G[z,k,ab,de] = sum_c basis[z,ab,k,c] * basis[z,de,k,c]      (tiny, host-computed)
  h[z,a,o]    = sum_{b,k,de} G[z,k,ab,de] * W[ab,de,k,o]      (device)

Two structural facts make this far smaller than the dense 256 MB W stream:

1. The RBF cutoff zeroes basis rows EXACTLY for atom pairs with dist >= 5
   (and for a==b, where diff==0). With x ~ N(0,9) per coord only ~20% of
   (ab, de) cells have sum_z |G[z,:,ab,de]| != 0, so ~80% of W is never
   touched. The host computes the exact activity mask from x and ships only
   the needed W[ab,de,:,:] cells. This is exact sparsity, not approximation.

2. Under this axon client a dispatch is wire-bound (~45-60 MB/s tunnel), so
   W cells go over as int8 (symmetric linear quant, one global scale:
   rel-to-max err ~7e-3 vs the 2e-2 gate; fp8 e4m3 measures 2-3e-2). The
   scale folds into the host epilogue; the device matmuls integer-valued
   fp16 and PSUM accumulates fp32 exactly.

Packed layout: every needed (ab, de) cell contributes K=16 contraction rows
(one per k), each row = 64 int8 W values + an 8-wide fp16 G vector
(slot*4+z, slot = which of the core's two atoms the row belongs to; the
other slot is zero). Rows are tiled 128 to a matmul. Atoms are paired
greedy (largest+smallest row count) onto 8 cores; all cores run one SPMD
program sized to the max core (zero-padded tiles contribute nothing).

Device per core: stream W tiles in 8 int8 chunks (HWDGE, per-chunk
semaphores), DVE casts each chunk to fp16 into a 2-deep ping-pong buffer,
PE runs one accumulating matmul per tile: psum[8,64] += G[128,8].T @
Wf[128,64]; DVE copies psum to SBUF, sync engine DMAs out h[8,64] fp32.
Host scales by the quant scale, applies silu + fc, and scatters the 2
atoms per core into the full (B,N,O) output.
"""

import numpy as np

import concourse.bass as bass
from concourse import mybir
from concourse.bass_utils import run_bass_kernel_spmd

# ---------------------------------------------------------------- constants
B, N, K, H, O = 4, 16, 16, 64, 1
EPSILON = 1e-5
CUT_LO, CUT_HI = 0.0, 5.0
N_CORES = 8
N_CHUNKS = 8
SLOTS = 2                       # atoms per core
GW = SLOTS * B                  # G vector width (slot-major, z minor)

_nc_cache = {}


def _basis_host(x):
    """Replicates reference featurization in float64; returns (B, N*N, K, 3)."""
    x = x.astype(np.float64)
    diff = x[:, :, None, :] - x[:, None, :, :]                # (B,N,N,3)
    norm_sq = np.sum(diff * diff, axis=-1, keepdims=True) + EPSILON
    norm = np.sqrt(norm_sq)
    diffn = diff / norm_sq
    start = np.exp(-CUT_HI + CUT_LO)
    means = np.linspace(start, 1.0, K)
    betas = (2.0 / K * (1.0 - start)) ** -2
    alpha = 5.0 / (CUT_HI - CUT_LO)
    cutoff = 0.5 * (np.cos(np.pi * norm / CUT_HI) + 1.0) * (norm < CUT_HI)
    smear = cutoff * np.exp(-betas * (np.exp(alpha * (-norm + CUT_LO)) - means) ** 2)
    basis = smear[..., None] * diffn[..., None, :]            # (B,N,N,K,3)
    return basis.reshape(B, N * N, K, 3)


def _build_nc(nt8, ct):
    """One SPMD Bass program; nt8 = padded tile count, ct = tiles/chunk."""
    nc = bass.Bass(target_bir_lowering=False)
    w = nc.dram_tensor("w", [128, N_CHUNKS, ct * H], mybir.dt.int8,
                       kind="ExternalInput")
    g = nc.dram_tensor("g", [128, nt8 * GW], mybir.dt.float16,
                       kind="ExternalInput")
    h = nc.dram_tensor("h", [GW, H], mybir.dt.float32, kind="ExternalOutput")

    import contextlib
    with contextlib.ExitStack() as st:
        gt = st.enter_context(nc.sbuf_tensor(
            "gt", [128, nt8 * GW], mybir.dt.float16))
        wt8 = st.enter_context(nc.sbuf_tensor(
            "wt8", [128, N_CHUNKS, ct * H], mybir.dt.int8))
        wf = st.enter_context(nc.sbuf_tensor(
            "wf", [128, 2, ct * H], mybir.dt.float16))
        ot = st.enter_context(nc.sbuf_tensor("ot", [GW, H], mybir.dt.float32))
        ps = st.enter_context(nc.psum_tensor("ps", [GW, H], mybir.dt.float32))
        g_sem = st.enter_context(nc.semaphore("g_sem"))
        w_sems = [st.enter_context(nc.semaphore(f"w_sem{cc}"))
                  for cc in range(N_CHUNKS)]
        conv_sem = st.enter_context(nc.semaphore("conv_sem"))
        pe_prog = st.enter_context(nc.semaphore("pe_prog"))
        cp_sem = st.enter_context(nc.semaphore("cp_sem"))
        out_sem = st.enter_context(nc.semaphore("out_sem"))
        block = st.enter_context(nc.Block())

        @block.sync
        def _(sync):
            for cc in range(N_CHUNKS):
                # per-chunk semaphores: a shared counting sem can race
                # across the 16 SDMA engines
                sync.dma_start(wt8[:, cc, :], w[:, cc, :]).then_inc(
                    w_sems[cc], 16)
            sync.wait_ge(cp_sem, 1)
            sync.dma_start(h[:, :], ot[:, :]).then_inc(out_sem, 16)
            sync.wait_ge(out_sem, 16)

        @block.vector
        def _(vector):
            for cc in range(N_CHUNKS):
                vector.wait_ge(w_sems[cc], 16)
                if cc >= 2:
                    # ping-pong slot cc%2 is free once PE finished chunk cc-2
                    vector.wait_ge(pe_prog, cc - 1)
                vector.tensor_copy(
                    out=wf[:, cc % 2, :], in_=wt8[:, cc, :],
                ).then_inc(conv_sem, 1)
            vector.wait_ge(pe_prog, N_CHUNKS)
            vector.tensor_copy(out=ot[:, :], in_=ps[:, :]).then_inc(cp_sem, 1)

        @block.tensor
        def _(tensor):
            tensor.wait_ge(g_sem, 16)
            for cc in range(N_CHUNKS):
                tensor.wait_ge(conv_sem, cc + 1)
                for t in range(ct):
                    tile = cc * ct + t
                    mm = tensor.matmul(
                        ps[:, :],
                        gt[:, tile * GW:(tile + 1) * GW],
                        wf[:, cc % 2, t * H:(t + 1) * H],
                        start=(tile == 0),
                        stop=(tile == nt8 - 1),
                    )
                    if t == ct - 1:
                        mm.then_inc(pe_prog, 1)

        @block.gpsimd
        def _(gpsimd):
            # G load on the SWDGE path overlaps W chunk 0 on the HWDGE ring.
            gpsimd.dma_start(gt[:, :], g[:, :]).then_inc(g_sem, 16)
    return nc


def _get_nc(nt8, ct):
    key = (nt8, ct)
    if key not in _nc_cache:
        _nc_cache[key] = _build_nc(nt8, ct)
    return _nc_cache[key]


def _make_inputs(x, W):
    """Returns (in_maps, meta) where meta = (scale, pairs, nt8, ct)."""
    x = np.asarray(x)
    bf = _basis_host(x)                                       # (B, 256, K, 3)
    # G[z,k,i,j] via batched matmul: (z,k,i,c) @ (z,k,c,j)
    bkt = bf.transpose(0, 2, 1, 3)                            # (B, K, 256, 3)
    G = bkt @ bkt.transpose(0, 1, 3, 2)                       # (B, K, 256, 256)

    # exact activity mask from the cutoff (and the a==b zero-diff rows)
    xd = x.astype(np.float64)
    diff = xd[:, :, None, :] - xd[:, None, :, :]
    dist = np.sqrt((diff ** 2).sum(-1) + EPSILON)
    act = (dist < CUT_HI) & ~np.eye(N, dtype=bool)[None]      # (B, N, N)
    actf = act.reshape(B, N * N)
    need = np.zeros((N * N, N * N), dtype=bool)               # (ab, de)
    for z in range(B):
        need |= actf[z][:, None] & actf[z][None, :]

    Wv = np.asarray(W).reshape(N * N, N * N, K, H)

    # Row-level pruning: a contraction row is (ab, de, k); its G vector over
    # z has max magnitude below G_TH for the overwhelming majority of rows
    # (RBF tails decay like exp(-65*dx^2)), and those rows' contributions
    # are orders of magnitude below the int8 quantization floor.
    G_TH = 3e-4
    # per-atom row lists (ab = a*N + b, so atom-major already)
    atom_rows_idx = []
    for a in range(N):
        b_idx, de_idx = np.nonzero(need[a * N:(a + 1) * N])
        ab = a * N + b_idx
        gmag = np.abs(G[:, :, ab, de_idx]).max(0)             # (K, nc)
        k_i, c_i = np.nonzero(gmag > G_TH)
        atom_rows_idx.append((ab[c_i], de_idx[c_i], k_i))
    rows_per_atom = np.array([len(r[0]) for r in atom_rows_idx])

    # pair largest with smallest onto the 8 cores
    order = np.argsort(-rows_per_atom, kind="stable")
    pairs = [(int(order[i]), int(order[2 * N_CORES - 1 - i]))
             for i in range(N_CORES)]
    core_rows = [rows_per_atom[i] + rows_per_atom[j] for i, j in pairs]
    nt = max((int(r) + 127) // 128 for r in core_rows) if max(core_rows) else 1
    ct = (nt + N_CHUNKS - 1) // N_CHUNKS
    nt8 = ct * N_CHUNKS

    # quantization scale over the kept rows only
    kept_w = [Wv[ab, de, k] for ab, de, k in atom_rows_idx]   # (nr, H) each
    wmax = max((float(np.abs(w).max()) for w in kept_w if w.size),
               default=1.0)
    scale = wmax / 127.0 if wmax > 0 else 1.0
    inv = 1.0 / scale

    def atom_rows(a):
        ab, de, k = atom_rows_idx[a]
        wc = np.rint(kept_w[a].astype(np.float64) * inv)      # (nr, H)
        np.clip(wc, -127, 127, out=wc)
        wr = wc.astype(np.int8)
        gr = G[:, k, ab, de].T                                # (nr, B)
        return wr, gr.astype(np.float16)

    in_maps = []
    for c in range(N_CORES):
        a0, a1 = pairs[c]
        w0, g0 = atom_rows(a0)
        w1, g1 = atom_rows(a1)
        nrows = nt8 * 128
        wrows = np.zeros((nrows, H), dtype=np.int8)
        grows = np.zeros((nrows, GW), dtype=np.float16)
        wrows[:len(w0)] = w0
        wrows[len(w0):len(w0) + len(w1)] = w1
        grows[:len(g0), 0:B] = g0
        grows[len(g0):len(g0) + len(g1), B:2 * B] = g1
        wc = wrows.reshape(nt8, 128, H).transpose(1, 0, 2).reshape(
            128, N_CHUNKS, ct * H)
        gc = grows.reshape(nt8, 128, GW).transpose(1, 0, 2).reshape(
            128, nt8 * GW)
        in_maps.append({
            "w": np.ascontiguousarray(wc),
            "g": np.ascontiguousarray(gc),
        })
    return in_maps, (scale, pairs, nt8, ct)


def kernel(x, W, w_fc, b_fc):
    in_maps, (scale, pairs, nt8, ct) = _make_inputs(x, W)
    nc = _get_nc(nt8, ct)
    res = run_bass_kernel_spmd(nc, in_maps, list(range(N_CORES))).results
    h = np.zeros((B, N, H), dtype=np.float64)
    for c in range(N_CORES):
        hc = res[c]["h"].reshape(SLOTS, B, H)                 # (slot, z, H)
        for s in range(SLOTS):
            h[:, pairs[c][s], :] = hc[s]
    h *= scale
    sil = h / (1.0 + np.exp(-h))
    out = sil @ w_fc.astype(np.float64) + b_fc.astype(np.float64)
    return out.astype(np.float32)


# revision 5
# speedup vs baseline: 19.3767x; 2.7705x over previous
"""Trainium2 Bass kernel for nn_JunmaiLayer (gnn_message_passing).

Math: h[z,a,o] = sum_{b,d,e,k,c} basis[z,a,b,k,c] * basis[z,d,e,k,c] * W[a,b,d,e,k,o]
      out = silu(h) @ w_fc + b_fc

Factoring:
  G[z,k,ab,de] = sum_c basis[z,ab,k,c] * basis[z,de,k,c]      (tiny, host-computed)
  h[z,a,o]    = sum_{b,k,de} G[z,k,ab,de] * W[ab,de,k,o]      (device)

Two structural facts make this far smaller than the dense 256 MB W stream:

1. The RBF cutoff zeroes basis rows EXACTLY for atom pairs with dist >= 5
   (and for a==b, where diff==0). With x ~ N(0,9) per coord only ~20% of
   (ab, de) cells have sum_z |G[z,:,ab,de]| != 0, so ~80% of W is never
   touched. The host computes the exact activity mask from x and ships only
   the needed W[ab,de,:,:] cells. This is exact sparsity, not approximation.

2. Under this axon client a dispatch is wire-bound (~45-60 MB/s tunnel), so
   W cells go over as int8 (symmetric linear quant, one global scale:
   rel-to-max err ~7e-3 vs the 2e-2 gate; fp8 e4m3 measures 2-3e-2). The
   scale folds into the host epilogue; the device matmuls integer-valued
   fp16 and PSUM accumulates fp32 exactly.

Packed layout: every needed (ab, de) cell contributes K=16 contraction rows
(one per k), each row = 64 int8 W values + an 8-wide fp16 G vector
(slot*4+z, slot = which of the core's two atoms the row belongs to; the
other slot is zero). Rows are tiled 128 to a matmul. Atoms are paired
greedy (largest+smallest row count) onto 8 cores; all cores run one SPMD
program sized to the max core (zero-padded tiles contribute nothing).

Device per core: stream W tiles in 8 int8 chunks (HWDGE, per-chunk
semaphores), DVE casts each chunk to fp16 into a 2-deep ping-pong buffer,
PE runs one accumulating matmul per tile: psum[8,64] += G[128,8].T @
Wf[128,64]; DVE copies psum to SBUF, sync engine DMAs out h[8,64] fp32.
Host scales by the quant scale, applies silu + fc, and scatters the 2
atoms per core into the full (B,N,O) output.
"""

import os
import tempfile

import numpy as np

import jax

# Persistent XLA compilation cache: run_bass_kernel_spmd re-jits a fresh
# wrapper every call, so without this each dispatch pays ~130 ms of XLA
# compile even though the NEFF itself is disk-cached.
try:
    jax.config.update(
        "jax_compilation_cache_dir",
        os.path.join(tempfile.gettempdir(), "jax_cc_cache"))
    jax.config.update("jax_persistent_cache_min_compile_time_secs", 0)
    jax.config.update("jax_persistent_cache_min_entry_size_bytes", 0)
except Exception:
    pass

import concourse.bass as bass
from concourse import mybir
from concourse.bass_utils import run_bass_kernel_spmd

# ---------------------------------------------------------------- constants
B, N, K, H, O = 4, 16, 16, 64, 1
EPSILON = 1e-5
CUT_LO, CUT_HI = 0.0, 5.0
N_CORES = 8
N_CHUNKS = 8
SLOTS = 2                       # atoms per core
GW = SLOTS * B                  # G vector width (slot-major, z minor)

_nc_cache = {}


def _basis_host(x):
    """Replicates reference featurization in float64; returns (B, N*N, K, 3)."""
    x = x.astype(np.float64)
    diff = x[:, :, None, :] - x[:, None, :, :]                # (B,N,N,3)
    norm_sq = np.sum(diff * diff, axis=-1, keepdims=True) + EPSILON
    norm = np.sqrt(norm_sq)
    diffn = diff / norm_sq
    start = np.exp(-CUT_HI + CUT_LO)
    means = np.linspace(start, 1.0, K)
    betas = (2.0 / K * (1.0 - start)) ** -2
    alpha = 5.0 / (CUT_HI - CUT_LO)
    cutoff = 0.5 * (np.cos(np.pi * norm / CUT_HI) + 1.0) * (norm < CUT_HI)
    smear = cutoff * np.exp(-betas * (np.exp(alpha * (-norm + CUT_LO)) - means) ** 2)
    basis = smear[..., None] * diffn[..., None, :]            # (B,N,N,K,3)
    return basis.reshape(B, N * N, K, 3)


def _build_nc(nt8, ct, chunks):
    """One SPMD Bass program; nt8 = chunks*ct padded tile count."""
    nc = bass.Bass(target_bir_lowering=False)
    w = nc.dram_tensor("w", [128, chunks, ct * H], mybir.dt.int8,
                       kind="ExternalInput")
    g = nc.dram_tensor("g", [128, nt8 * GW], mybir.dt.float16,
                       kind="ExternalInput")
    h = nc.dram_tensor("h", [GW, H], mybir.dt.float32, kind="ExternalOutput")

    import contextlib
    with contextlib.ExitStack() as st:
        gt = st.enter_context(nc.sbuf_tensor(
            "gt", [128, nt8 * GW], mybir.dt.float16))
        wt8 = st.enter_context(nc.sbuf_tensor(
            "wt8", [128, chunks, ct * H], mybir.dt.int8))
        wf = st.enter_context(nc.sbuf_tensor(
            "wf", [128, 2, ct * H], mybir.dt.float16))
        ot = st.enter_context(nc.sbuf_tensor("ot", [GW, H], mybir.dt.float32))
        ps = st.enter_context(nc.psum_tensor("ps", [GW, H], mybir.dt.float32))
        g_sem = st.enter_context(nc.semaphore("g_sem"))
        w_sems = [st.enter_context(nc.semaphore(f"w_sem{cc}"))
                  for cc in range(chunks)]
        conv_sem = st.enter_context(nc.semaphore("conv_sem"))
        pe_prog = st.enter_context(nc.semaphore("pe_prog"))
        cp_sem = st.enter_context(nc.semaphore("cp_sem"))
        out_sem = st.enter_context(nc.semaphore("out_sem"))
        block = st.enter_context(nc.Block())

        @block.sync
        def _(sync):
            for cc in range(chunks):
                # per-chunk semaphores: a shared counting sem can race
                # across the 16 SDMA engines
                sync.dma_start(wt8[:, cc, :], w[:, cc, :]).then_inc(
                    w_sems[cc], 16)
            sync.wait_ge(cp_sem, 1)
            sync.dma_start(h[:, :], ot[:, :]).then_inc(out_sem, 16)
            sync.wait_ge(out_sem, 16)

        @block.vector
        def _(vector):
            for cc in range(chunks):
                vector.wait_ge(w_sems[cc], 16)
                if cc >= 2:
                    # ping-pong slot cc%2 is free once PE finished chunk cc-2
                    vector.wait_ge(pe_prog, cc - 1)
                vector.tensor_copy(
                    out=wf[:, cc % 2, :], in_=wt8[:, cc, :],
                ).then_inc(conv_sem, 1)
            vector.wait_ge(pe_prog, chunks)
            vector.tensor_copy(out=ot[:, :], in_=ps[:, :]).then_inc(cp_sem, 1)

        @block.tensor
        def _(tensor):
            tensor.wait_ge(g_sem, 16)
            for cc in range(chunks):
                tensor.wait_ge(conv_sem, cc + 1)
                for t in range(ct):
                    tile = cc * ct + t
                    mm = tensor.matmul(
                        ps[:, :],
                        gt[:, tile * GW:(tile + 1) * GW],
                        wf[:, cc % 2, t * H:(t + 1) * H],
                        start=(tile == 0),
                        stop=(tile == nt8 - 1),
                    )
                    if t == ct - 1:
                        mm.then_inc(pe_prog, 1)

        @block.gpsimd
        def _(gpsimd):
            # G load on the SWDGE path overlaps W chunk 0 on the HWDGE ring.
            gpsimd.dma_start(gt[:, :], g[:, :]).then_inc(g_sem, 16)
    return nc


def _get_nc(key):
    if key not in _nc_cache:
        _nc_cache[key] = _build_nc(*key)
    return _nc_cache[key]


def _make_inputs(x, W):
    """Returns (in_maps, meta) where meta = (scale, pairs, nt8, ct)."""
    x = np.asarray(x)
    bf = _basis_host(x)                                       # (B, 256, K, 3)
    # G[z,k,i,j] via batched matmul: (z,k,i,c) @ (z,k,c,j)
    bkt = bf.transpose(0, 2, 1, 3)                            # (B, K, 256, 3)
    G = bkt @ bkt.transpose(0, 1, 3, 2)                       # (B, K, 256, 256)

    # exact activity mask from the cutoff (and the a==b zero-diff rows)
    xd = x.astype(np.float64)
    diff = xd[:, :, None, :] - xd[:, None, :, :]
    dist = np.sqrt((diff ** 2).sum(-1) + EPSILON)
    act = (dist < CUT_HI) & ~np.eye(N, dtype=bool)[None]      # (B, N, N)
    actf = act.reshape(B, N * N)
    need = np.zeros((N * N, N * N), dtype=bool)               # (ab, de)
    for z in range(B):
        need |= actf[z][:, None] & actf[z][None, :]

    Wv = np.asarray(W).reshape(N * N, N * N, K, H)

    # Row-level pruning: a contraction row is (ab, de, k); its G vector over
    # z has max magnitude below G_TH for the overwhelming majority of rows
    # (RBF tails decay like exp(-65*dx^2)), and those rows' contributions
    # are orders of magnitude below the int8 quantization floor.
    G_TH = 1e-3
    # per-atom row lists (ab = a*N + b, so atom-major already)
    atom_rows_idx = []
    for a in range(N):
        b_idx, de_idx = np.nonzero(need[a * N:(a + 1) * N])
        ab = a * N + b_idx
        gmag = np.abs(G[:, :, ab, de_idx]).max(0)             # (K, nc)
        k_i, c_i = np.nonzero(gmag > G_TH)
        atom_rows_idx.append((ab[c_i], de_idx[c_i], k_i))
    rows_per_atom = np.array([len(r[0]) for r in atom_rows_idx])

    # pair largest with smallest onto the 8 cores
    order = np.argsort(-rows_per_atom, kind="stable")
    pairs = [(int(order[i]), int(order[2 * N_CORES - 1 - i]))
             for i in range(N_CORES)]
    core_rows = [rows_per_atom[i] + rows_per_atom[j] for i, j in pairs]
    nt = max((int(r) + 127) // 128 for r in core_rows) if max(core_rows) else 1
    ct = (nt + N_CHUNKS - 1) // N_CHUNKS
    chunks = (nt + ct - 1) // ct
    nt8 = ct * chunks

    # quantization scale over the kept rows only
    kept_w = [Wv[ab, de, k] for ab, de, k in atom_rows_idx]   # (nr, H) each
    wmax = max((float(np.abs(w).max()) for w in kept_w if w.size),
               default=1.0)
    scale = wmax / 127.0 if wmax > 0 else 1.0
    inv = 1.0 / scale

    def atom_rows(a):
        ab, de, k = atom_rows_idx[a]
        wc = np.rint(kept_w[a].astype(np.float64) * inv)      # (nr, H)
        np.clip(wc, -127, 127, out=wc)
        wr = wc.astype(np.int8)
        gr = G[:, k, ab, de].T                                # (nr, B)
        return wr, gr.astype(np.float16)

    in_maps = []
    for c in range(N_CORES):
        a0, a1 = pairs[c]
        w0, g0 = atom_rows(a0)
        w1, g1 = atom_rows(a1)
        nrows = nt8 * 128
        wrows = np.zeros((nrows, H), dtype=np.int8)
        grows = np.zeros((nrows, GW), dtype=np.float16)
        wrows[:len(w0)] = w0
        wrows[len(w0):len(w0) + len(w1)] = w1
        grows[:len(g0), 0:B] = g0
        grows[len(g0):len(g0) + len(g1), B:2 * B] = g1
        wc = wrows.reshape(nt8, 128, H).transpose(1, 0, 2).reshape(
            128, chunks, ct * H)
        gc = grows.reshape(nt8, 128, GW).transpose(1, 0, 2).reshape(
            128, nt8 * GW)
        in_maps.append({
            "w": np.ascontiguousarray(wc),
            "g": np.ascontiguousarray(gc),
        })
    return in_maps, (scale, pairs, (nt8, ct, chunks))


def kernel(x, W, w_fc, b_fc):
    in_maps, (scale, pairs, prog_key) = _make_inputs(x, W)
    nc = _get_nc(prog_key)
    res = run_bass_kernel_spmd(nc, in_maps, list(range(N_CORES))).results
    h = np.zeros((B, N, H), dtype=np.float64)
    for c in range(N_CORES):
        hc = res[c]["h"].reshape(SLOTS, B, H)                 # (slot, z, H)
        for s in range(SLOTS):
            h[:, pairs[c][s], :] = hc[s]
    h *= scale
    sil = h / (1.0 + np.exp(-h))
    out = sil @ w_fc.astype(np.float64) + b_fc.astype(np.float64)
    return out.astype(np.float32)
